# revision 2
# baseline (speedup 1.0000x reference)
"""Trainium2 Bass kernel for nn_Attention_63118839382659 (gnn_message_passing).

Math (derived from the reference):
  g[b,t,k,l] = (q1*k1)[b,t,k] * (q2*k2)[b,t,l]   -- rank-1 per token
  u = q1*k1, v = q2*k2                            [B,T,R]
  M_j[b]  = u_j[b]^T v_j[b] / T                   [R,R]
  P_j     = M_l1 @ M_l2  (l1<l2, l!=j)
  w_j     = v_j @ P_j
  out_j   = ((u_j (x) w_j) @ Wa_j + beta) * x_j

Sharding: pure data-parallel over batch, 4 batches/core on 8 cores, no
collectives.  Everything runs in a transposed layout (feature dim on SBUF
partitions): the host feeds x pre-transposed per (modality, batch) and
un-transposes the output, so the device never transposes x.

Compute dtype bf16 (fp32 matmul is ~4x slower on the PE); PSUM accumulation
fp32.  Validated numpy bf16 pipeline rel err ~3.4e-3 (gate 2e-2).
"""

import numpy as np
import ml_dtypes

B, T, D, R, NM = 32, 512, 512, 32, 3
BETA = 0.5
NCORES = 8
BL = B // NCORES          # batches per core = 4
DC = D // 128             # 4 d-chunks
RRC = (R * R) // 128      # 8 rr-chunks

BF16 = ml_dtypes.bfloat16

_CACHE = {}


def _split_excess_waits(nc, max_waits=1):
    """walrus in this container rejects >1 semaphore wait per instruction
    (CTRL_NO_STRUCT setupSyncWait). Split extras onto preceding NoOps."""
    import concourse.mybir as mybir
    n = 0
    for fn in nc.m.functions:
        for bb in fn.blocks:
            new = []
            for inst in bb.instructions:
                si = getattr(inst, "sync_info", None)
                waits = list(si.on_wait) if (si is not None and si.on_wait) else []
                if len(waits) > max_waits:
                    excess, keep = waits[:-max_waits], waits[-max_waits:]
                    for i in range(0, len(excess), max_waits):
                        new.append(mybir.InstNoOp(
                            name=f"{inst.name}-ws{i}",
                            engine=inst.engine,
                            bass_nofuse=True,
                            sync_info=mybir.SyncInfo(
                                on_wait=excess[i:i + max_waits], on_update=[]),
                        ))
                    si.on_wait = keep
                    n += 1
                new.append(inst)
            bb.instructions[:] = new
    return n


def build_nc():
    import concourse.bass as bass
    import concourse.mybir as mybir
    from concourse.bass import ts, ds
    from concourse.tile import TileContext

    bf = mybir.dt.bfloat16
    f32 = mybir.dt.float32

    nc = bass.Bass()
    xt_e = nc.declare_dram_parameter("xt", [NM, BL, 128, DC, T], bf, isOutput=False)
    wall_e = nc.declare_dram_parameter("wallh", [NM, 128, DC, 128], bf, isOutput=False)
    wa_e = nc.declare_dram_parameter("wah", [NM, 128, RRC, T], bf, isOutput=False)
    sm_e = nc.declare_dram_parameter("smats", [R, RRC, 128], bf, isOutput=False)
    s4_e = nc.declare_dram_parameter("s4", [R, 128], bf, isOutput=False)
    id_e = nc.declare_dram_parameter("ident", [64, 64], bf, isOutput=False)
    out_e = nc.declare_dram_parameter("outp", [NM, BL, 128, DC, T], bf, isOutput=True)

    with TileContext(nc) as tc:
        with (
            tc.tile_pool(name="wpool", bufs=1) as wpool,
            tc.tile_pool(name="xpool", bufs=6) as xpool,
            tc.tile_pool(name="uvpool", bufs=6) as uvpool,
            tc.tile_pool(name="uvnpool", bufs=3) as uvnpool,
            tc.tile_pool(name="mpool", bufs=10) as mpool,
            tc.tile_pool(name="wtpool", bufs=3) as wtpool,
            tc.tile_pool(name="wspool", bufs=2) as wspool,
            tc.tile_pool(name="uepool", bufs=3) as uepool,
            tc.tile_pool(name="outerpool", bufs=3) as outerpool,
            tc.tile_pool(name="opool", bufs=3) as opool,
            tc.tile_pool(name="ps_proj", bufs=1, space="PSUM") as ps_proj,
            tc.tile_pool(name="ps_small", bufs=2, space="PSUM") as ps_small,
            tc.tile_pool(name="ps_ue", bufs=2, space="PSUM") as ps_ue,
            tc.tile_pool(name="ps_ws", bufs=1, space="PSUM") as ps_ws,
            tc.tile_pool(name="ps_acc", bufs=2, space="PSUM") as ps_acc,
        ):
            # ---- resident weights/constants ----
            wall_sb, wa_sb = [], []
            for j in range(NM):
                wt = wpool.tile([128, DC, 128], bf, name=f"wall{j}")
                nc.sync.dma_start(out=wt[:], in_=wall_e[j])
                wall_sb.append(wt)
                at = wpool.tile([128, RRC, T], bf, name=f"wa{j}")
                nc.sync.dma_start(out=at[:], in_=wa_e[j])
                wa_sb.append(at)
            sm_sb = wpool.tile([R, RRC, 128], bf, name="smats")
            nc.sync.dma_start(out=sm_sb[:], in_=sm_e[:])
            s4_sb = wpool.tile([R, 128], bf, name="s4")
            nc.sync.dma_start(out=s4_sb[:], in_=s4_e[:])
            id_sb = wpool.tile([64, 64], bf, name="ident")
            nc.sync.dma_start(out=id_sb[:], in_=id_e[:])

            for b in range(BL):
                xsb = {}
                uv = {}
                msb = {}   # (j, 'n'|'t') -> [32,32] bf16 sbuf
                # ---------- phase A: proj, u/v, M ----------
                for j in range(NM):
                    xt = xpool.tile([128, DC, T], bf, name=f"x_{j}_{b}", tag="xt")
                    nc.sync.dma_start(out=xt[:], in_=xt_e[j, b])
                    xsb[j] = xt

                    qk = ps_proj.tile([128, T], f32, name=f"qk_{j}_{b}", tag="qk")
                    for c in range(DC):
                        nc.tensor.matmul(qk[:], wall_sb[j][:, c, :], xt[:, c, :],
                                         start=(c == 0), stop=(c == DC - 1))
                    kk = uvnpool.tile([64, T], bf, name=f"kk_{j}_{b}", tag="kk")
                    nc.scalar.copy(kk[:], qk[64:128, :])
                    uvt = uvpool.tile([64, T], bf, name=f"uv_{j}_{b}", tag="uv")
                    nc.vector.tensor_mul(uvt[:], qk[0:64, :], kk[:])
                    uv[j] = uvt

                    uvn = uvnpool.tile([128, DC, 64], bf, name=f"uvn_{j}_{b}", tag="uvn")
                    for tq in range(DC):
                        trp = ps_small.tile([128, 64], bf, name=f"tr_{j}_{b}_{tq}", tag="sm")
                        nc.tensor.transpose(trp[:], uvt[:, ts(tq, 128)], id_sb[:])
                        nc.scalar.copy(uvn[:, tq, :], trp[:])

                    # M_j / M_j^T (only the ones needed):
                    #   P0 = M1 @ M2 -> lhsT=M1T rhs=M2
                    #   P1 = M0 @ M2 -> lhsT=M0T rhs=M2
                    #   P2 = M0 @ M1 -> lhsT=M0T rhs=M1
                    need_n = j in (1, 2)    # M1, M2 used as rhs
                    need_t = j in (0, 1)    # M0T, M1T used as lhsT
                    if need_n:
                        mp = ps_small.tile([R, R], f32, name=f"m_{j}_{b}", tag="sm")
                        for tq in range(DC):
                            nc.tensor.matmul(mp[:], uvn[:, tq, 0:32], uvn[:, tq, 32:64],
                                             start=(tq == 0), stop=(tq == DC - 1))
                        ms = mpool.tile([R, R], bf, name=f"ms_{j}_{b}", tag="ms")
                        nc.scalar.mul(ms[:], mp[:], 1.0 / T)
                        msb[(j, 'n')] = ms
                    if need_t:
                        mtp = ps_small.tile([R, R], f32, name=f"mt_{j}_{b}", tag="sm")
                        for tq in range(DC):
                            nc.tensor.matmul(mtp[:], uvn[:, tq, 32:64], uvn[:, tq, 0:32],
                                             start=(tq == 0), stop=(tq == DC - 1))
                        mts = mpool.tile([R, R], bf, name=f"mts_{j}_{b}", tag="ms")
                        nc.scalar.mul(mts[:], mtp[:], 1.0 / T)
                        msb[(j, 't')] = mts

                # ---------- phase B: P, w, outer, big matmul, epilogue ----------
                for j in range(NM):
                    l1, l2 = [l for l in range(NM) if l != j]
                    pp = ps_small.tile([R, R], f32, name=f"p_{j}_{b}", tag="sm")
                    nc.tensor.matmul(pp[:], msb[(l1, 't')][:], msb[(l2, 'n')][:],
                                     start=True, stop=True)
                    psb = mpool.tile([R, R], bf, name=f"ps_{j}_{b}", tag="ms")
                    nc.scalar.copy(psb[:], pp[:])

                    # v at base-partition 0 for the w matmul rhs
                    v0 = wtpool.tile([R, T], bf, name=f"v0_{j}_{b}", tag="wt")
                    nc.scalar.copy(v0[:], uv[j][32:64, :])

                    wtp = ps_small.tile([R, T], f32, name=f"wtp_{j}_{b}", tag="sm")
                    nc.tensor.matmul(wtp[:], psb[:], v0[:], start=True, stop=True)
                    wts = wtpool.tile([R, T], bf, name=f"wts_{j}_{b}", tag="wt")
                    nc.scalar.copy(wts[:], wtp[:])

                    wsp = ps_ws.tile([128, T], f32, name=f"wsp_{j}_{b}", tag="ws")
                    nc.tensor.matmul(wsp[:], s4_sb[:], wts[:], start=True, stop=True)
                    wss = wspool.tile([128, T], bf, name=f"wss_{j}_{b}", tag="ws")
                    nc.scalar.copy(wss[:], wsp[:])

                    outer = outerpool.tile([128, RRC, T], bf, name=f"outer_{j}_{b}", tag="outer")
                    for c in range(RRC):
                        uep = ps_ue.tile([128, T], f32, name=f"uep_{j}_{b}_{c}", tag="ue")
                        nc.tensor.matmul(uep[:], sm_sb[:, c, :], uv[j][0:32, :],
                                         start=True, stop=True)
                        ues = uepool.tile([128, T], bf, name=f"ues_{j}_{b}_{c}", tag="ue")
                        nc.scalar.copy(ues[:], uep[:])
                        nc.vector.tensor_mul(outer[:, c, :], wss[:], ues[:])

                    osb = opool.tile([128, DC, T], bf, name=f"o_{j}_{b}", tag="o")
                    for dt in range(DC):
                        acc = ps_acc.tile([128, T], f32, name=f"acc_{j}_{b}_{dt}", tag="acc")
                        for c in range(RRC):
                            nc.tensor.matmul(acc[:], wa_sb[j][:, c, ts(dt, 128)],
                                             outer[:, c, :],
                                             start=(c == 0), stop=(c == RRC - 1))
                        nc.vector.scalar_tensor_tensor(
                            osb[:, dt, :], acc[:], BETA, xsb[j][:, dt, :],
                            mybir.AluOpType.add, mybir.AluOpType.mult)
                    nc.sync.dma_start(out=out_e[j, b], in_=osb[:])

    _split_excess_waits(nc)
    return nc


def _consts():
    smats = np.zeros((R, RRC, 128), dtype=BF16)
    for c in range(RRC):
        for p in range(128):
            smats[4 * c + p // 32, c, p] = 1
    s4 = np.zeros((R, 128), dtype=BF16)
    for p in range(128):
        s4[p % 32, p] = 1
    ident = np.eye(64, dtype=BF16)
    return smats, s4, ident


def kernel(x0, x1, x2, Wq1, bq1, Wq2, bq2, Wk1, bk1, Wk2, bk2, Wa, ba):
    from concourse.bass_utils import run_bass_kernel_spmd

    if "nc" not in _CACHE:
        _CACHE["nc"] = build_nc()
    nc = _CACHE["nc"]

    x = np.stack([x0, x1, x2]).astype(np.float32)          # [3,B,T,D]
    # xt[j,b,p,c,t] = x[j,b,t,128c+p]
    xt = np.ascontiguousarray(
        x.transpose(0, 1, 3, 2)                             # [3,B,D,T]
         .reshape(NM, B, DC, 128, T)
         .transpose(0, 1, 3, 2, 4)).astype(BF16)            # [3,B,128,DC,T]
    wall = np.concatenate([Wq1, Wq2, Wk1, Wk2], axis=2)     # [3,512,128]
    wallh = np.ascontiguousarray(
        wall.reshape(NM, DC, 128, 128).transpose(0, 2, 1, 3)).astype(BF16)
    wah = np.ascontiguousarray(
        np.asarray(Wa).reshape(NM, RRC, 128, D).transpose(0, 2, 1, 3)).astype(BF16)
    smats, s4, ident = _consts()

    shared = {"wallh": wallh, "wah": wah, "smats": smats, "s4": s4, "ident": ident}
    in_maps = [
        {"xt": np.ascontiguousarray(xt[:, i * BL:(i + 1) * BL]), **shared}
        for i in range(NCORES)
    ]
    res = run_bass_kernel_spmd(nc, in_maps, core_ids=list(range(NCORES)))

    out = np.empty((NM, B, T, D), dtype=np.float32)
    for i in range(NCORES):
        o = np.asarray(res.results[i]["outp"]).astype(np.float32)  # [3,BL,128,DC,T]
        # out[j, b, t, 128*dt+p] = o[j, bl, p, dt, t]
        out[:, i * BL:(i + 1) * BL] = o.transpose(0, 1, 4, 3, 2).reshape(NM, BL, T, D)
    return tuple(out[j] for j in range(NM))
